# revision 1
# baseline (speedup 1.0000x reference)
"""CVQNN batched policy forward on 8 Trainium2 NeuronCores.

Math: B=256 4-mode Fock states (cutoff 10) through 4 CVQNN layers.
Device layout per core: state [100 partitions = two modes, 6400 free =
(plane, group, mode_k, mode_l, b16)], complex as separate re/im planes.

Every gate is a 100x100 complex matmul on the partition-side mode pair
(4 float32r matmuls with PSUM accumulation). Diagonal gates (phases,
Kerr) and single-mode gates (squeeze, displacement) are folded into
neighboring pair gates on the host. Pairing changes are strided
SBUF->SBUF DMAs (partition-contiguous by construction); orientation
flips are PE transposes. Batch is data-parallel over 8 cores, and split
into 2 groups per core so shuffles overlap gate matmuls.
"""
import numpy as np

B, D, CUT, LAYERS, NCORES = 256, 4, 10, 4, 8
K = D * (D - 1) // 2
C2 = CUT * CUT
BC = B // NCORES            # 32 batch per core
NG = 2                      # groups per core
BG = BC // NG               # 16
GF = C2 * BG                # 1600 free elems per (plane, group)
PF = NG * GF                # 3200 per plane
FTOT = 2 * PF               # 6400 per state tile
NMM = 12 * LAYERS + 1       # 49 pair-gate matmuls

# ---------------------------------------------------------------- host math

def _ops():
    a = np.diag(np.sqrt(np.arange(1, CUT)), k=1).astype(np.complex128)
    return a, a.conj().T.copy(), np.arange(CUT, dtype=np.float64)


def _expm_antiherm(G):
    w, V = np.linalg.eigh(-1j * G)
    return (V * np.exp(1j * w)) @ V.conj().T


def _bs(a, adag, t, p):
    Aab = np.kron(adag, a)
    return _expm_antiherm(t * (np.exp(1j * p) * Aab - np.exp(-1j * p) * Aab.conj().T))


def _orient(M, pair, in_order, out_order):
    """M acts canonically on index m_x*10+m_y for pair=(x,y).  Reindex for
    input digit order in_order and output digit order out_order."""
    M4 = M.reshape(CUT, CUT, CUT, CUT)  # [ox, oy, ix, iy]
    perm = [0, 1, 2, 3]
    if tuple(out_order) != tuple(pair):
        perm[0], perm[1] = perm[1], perm[0]
    if tuple(in_order) != tuple(pair):
        perm[2], perm[3] = perm[3], perm[2]
    return M4.transpose(perm).reshape(C2, C2)


def build_gates(cvqnn_weights):
    """Returns (mm_list, layout_final) where mm_list entries are
    (matrix[100x100 complex], swap_free: bool). Layout walk is fixed."""
    a, adag, n = _ops()
    I10 = np.eye(CUT, dtype=np.complex128)
    w = np.asarray(cvqnn_weights, np.float64)
    PAIRS = [(0, 1), (0, 2), (0, 3), (1, 2), (1, 3), (2, 3)]

    mms = []
    fold01 = np.eye(C2, dtype=np.complex128)
    fold2 = I10.copy()
    fold3 = I10.copy()
    for l in range(LAYERS):
        o = 0
        th1 = w[l, o:o + K]; o += K
        ph1 = w[l, o:o + K]; o += K
        vp1 = w[l, o:o + D]; o += D
        rsq = w[l, o:o + D]; o += D
        th2 = w[l, o:o + K]; o += K
        ph2 = w[l, o:o + K]; o += K
        vp2 = w[l, o:o + D]; o += D
        rd = w[l, o:o + D]; o += D
        phd = w[l, o:o + D]; o += D
        kap = w[l, o:o + D]
        U = {PAIRS[q]: _bs(a, adag, th1[q], ph1[q]) for q in range(K)}
        V = {PAIRS[q]: _bs(a, adag, th2[q], ph2[q]) for q in range(K)}
        S = [_expm_antiherm(0.5 * rsq[m] * (a @ a - adag @ adag)) for m in range(D)]
        al = rd * np.exp(1j * phd)
        Dm = [_expm_antiherm(al[m] * adag - np.conj(al[m]) * a) for m in range(D)]
        P1 = [np.diag(np.exp(1j * vp1[m] * n)) for m in range(D)]
        P2 = [np.diag(np.exp(1j * vp2[m] * n)) for m in range(D)]
        Km = [np.diag(np.exp(1j * kap[m] * n * n)) for m in range(D)]
        SQ01 = np.kron(S[0] @ P1[0], S[1] @ P1[1])
        SQ23 = np.kron(S[2] @ P1[2], S[3] @ P1[3])
        DP01 = np.kron(Dm[0] @ P2[0], Dm[1] @ P2[1])
        DP23 = np.kron(Dm[2] @ P2[2], Dm[3] @ P2[3])

        mms.append((_orient(U[(0, 1)] @ fold01, (0, 1), (1, 0), (1, 0)), False))
        mms.append((_orient(U[(0, 2)] @ np.kron(I10, fold2), (0, 2), (0, 2), (2, 0)), True))
        mms.append((_orient(U[(0, 3)] @ np.kron(I10, fold3), (0, 3), (0, 3), (0, 3)), False))
        mms.append((_orient(U[(1, 2)], (1, 2), (2, 1), (2, 1)), False))
        mms.append((_orient(U[(1, 3)], (1, 3), (1, 3), (1, 3)), False))
        mms.append((_orient(SQ23 @ U[(2, 3)], (2, 3), (3, 2), (3, 2)), False))
        mms.append((_orient(V[(0, 1)] @ SQ01, (0, 1), (1, 0), (1, 0)), False))
        mms.append((_orient(V[(0, 2)], (0, 2), (0, 2), (2, 0)), True))
        mms.append((_orient(V[(0, 3)], (0, 3), (0, 3), (0, 3)), False))
        mms.append((_orient(V[(1, 2)], (1, 2), (2, 1), (2, 1)), False))
        mms.append((_orient(V[(1, 3)], (1, 3), (1, 3), (1, 3)), False))
        mms.append((_orient(DP23 @ V[(2, 3)], (2, 3), (3, 2), (3, 2)), False))
        if l < LAYERS - 1:
            fold01 = np.kron(Km[0], Km[1]) @ DP01
            fold2 = Km[2]
            fold3 = Km[3]
        else:
            mms.append((_orient(DP01, (0, 1), (1, 0), (1, 0)), False))
    assert len(mms) == NMM
    return mms


# op schedule per layer: 'M' gate, 'S' shuffle, 'F' flip
LAYER_OPS = ['M', 'S', 'M', 'S', 'M', 'F', 'M', 'S', 'M', 'S', 'M', 'F',
             'M', 'S', 'M', 'S', 'M', 'F', 'M', 'S', 'M', 'S', 'M', 'F']
FULL_OPS = LAYER_OPS * LAYERS + ['M']


def initial_state_dev(inputs):
    """Per-core device state tiles [NCORES, 100, 6400] float32 in layout
    [1,0 | 2,3]: p=m1*10+m0, f=plane*3200+g*1600+m2*160+m3*16+b."""
    a, adag, n = _ops()
    z = 0.5j
    S0 = _expm_antiherm(0.5 * (np.conj(z) * (a @ a) - z * (adag @ adag)))
    psi0 = S0[:, 0]
    r = np.asarray(inputs, np.float64).reshape(-1)
    wv, Vx = np.linalg.eigh(-1j * (adag - a))
    w0 = Vx.conj().T @ psi0
    psi = (np.exp(1j * np.outer(r, wv)) * w0[None, :]) @ Vx.T
    psi = psi.reshape(B, D, CUT)
    st = np.einsum('bi,bj,bk,bl->bijkl', psi[:, 0], psi[:, 1], psi[:, 2], psi[:, 3])
    # [b, m0,m1,m2,m3] -> p=(m1,m0), f=(m2,m3,b16)
    st = st.transpose(2, 1, 3, 4, 0).reshape(C2, C2, B)  # [p(m1m0), (m2m3), b]
    out = np.empty((NCORES, C2, FTOT), np.float32)
    for c in range(NCORES):
        for g in range(NG):
            blk = st[:, :, c * BC + g * BG: c * BC + (g + 1) * BG]  # [100,100,16]
            blk = blk.reshape(C2, GF)
            out[c, :, g * GF:(g + 1) * GF] = blk.real.astype(np.float32)
            out[c, :, PF + g * GF:PF + (g + 1) * GF] = blk.imag.astype(np.float32)
    return out


def readout_weights():
    """lhsT [100, 4] for the device readout matmul, layout [1,0|2,3]."""
    n = np.arange(CUT, dtype=np.float32)
    Wt = np.zeros((C2, 4), np.float32)
    for p in range(C2):
        Wt[p, 0] = n[p % 10]    # mode 0 (partition minor)
        Wt[p, 1] = n[p // 10]   # mode 1 (partition major)
        Wt[p, 2] = 1.0
        Wt[p, 3] = 1.0
    return Wt


def assemble_output(routs):
    """routs: [NCORES, 4, 3200] -> [B, 4]."""
    n = np.arange(CUT, dtype=np.float64)
    f = np.arange(PF)
    w2 = n[(f % GF) // (CUT * BG)]   # mode2 digit weight
    w3 = n[(f % (CUT * BG)) // BG]   # mode3 digit weight
    out = np.zeros((B, D), np.float64)
    for c in range(NCORES):
        R = np.asarray(routs[c], np.float64)
        e0 = R[0].reshape(NG, C2, BG).sum(axis=1)            # [g, b]
        e1 = R[1].reshape(NG, C2, BG).sum(axis=1)
        e2 = (R[2] * w2).reshape(NG, C2, BG).sum(axis=1)
        e3 = (R[3] * w3).reshape(NG, C2, BG).sum(axis=1)
        for g in range(NG):
            sl = slice(c * BC + g * BG, c * BC + (g + 1) * BG)
            out[sl, 0] = e0[g]
            out[sl, 1] = e1[g]
            out[sl, 2] = e2[g]
            out[sl, 3] = e3[g]
    return out.astype(np.float32)


def gates_dram(mms):
    """[100, NMM*300] float32: per gate UrT | (-Ui)T | UiT, packed columns."""
    g = np.empty((C2, NMM * 3 * C2), np.float32)
    for i, (M, _) in enumerate(mms):
        g[:, i * 300:i * 300 + C2] = M.real.T.astype(np.float32)
        g[:, i * 300 + C2:i * 300 + 2 * C2] = (-M.imag.T).astype(np.float32)
        g[:, i * 300 + 2 * C2:i * 300 + 3 * C2] = M.imag.T.astype(np.float32)
    return g


# ------------------------------------------------------------ numpy dev-sim

def dev_sim(state_core, mms):
    """Bit-faithful numpy model of the device op stream for one core.
    state_core: [100, 6400] f32. Returns R [4, 3200] f32."""
    S = state_core.astype(np.float64)
    mi = 0
    for op in FULL_OPS:
        if op == 'M':
            M, swap = mms[mi]; mi += 1
            Sc = S[:, 0:PF] + 1j * S[:, PF:FTOT]
            Sc = M @ Sc
            if swap:
                Sc = Sc.reshape(C2, NG, CUT, CUT, BG).transpose(0, 1, 3, 2, 4).reshape(C2, PF)
            S = np.concatenate([Sc.real, Sc.imag], axis=1)
        elif op == 'S':
            # out[s*10+u, v*160+w*16+b] = in[v*10+s, u*160+w*16+b]
            X = S.reshape(C2, 2, NG, CUT, CUT, BG)
            X4 = X.reshape(CUT, CUT, 2, NG, CUT, CUT, BG)  # [v,s,pl,g,u,w,b]
            Y = X4.transpose(1, 4, 2, 3, 0, 5, 6)          # [s,u,pl,g,v,w,b]
            S = Y.reshape(C2, FTOT)
        else:  # flip
            X4 = S.reshape(CUT, CUT, 2, NG, CUT, CUT, BG)  # [p1,p2,pl,g,f1,f2,b]
            Y = X4.transpose(4, 5, 2, 3, 1, 0, 6)          # [f1,f2,pl,g,p2,p1,b]
            S = Y.reshape(C2, FTOT)
    P = S[:, 0:PF] ** 2 + S[:, PF:FTOT] ** 2
    Wt = readout_weights().astype(np.float64)
    return (Wt.T @ P).astype(np.float32)


# ------------------------------------------------------------- bass program

_NC_CACHE = {}


def build_bass(repeats=1):
    if repeats in _NC_CACHE:
        return _NC_CACHE[repeats]
    import concourse.bass as bass
    import concourse.mybir as mybir
    from concourse.tile import TileContext
    F32 = mybir.dt.float32
    F32R = mybir.dt.float32r

    nc = bass.Bass()
    d_state = nc.dram_tensor("state0", [C2, FTOT], F32, kind="ExternalInput")
    d_gates = nc.dram_tensor("gates", [C2, NMM * 3 * C2], F32, kind="ExternalInput")
    d_ident = nc.dram_tensor("ident", [C2, C2], F32, kind="ExternalInput")
    d_wread = nc.dram_tensor("wread", [C2, 4], F32, kind="ExternalInput")
    d_rout = nc.dram_tensor("rout", [4, PF], F32, kind="ExternalOutput")

    with TileContext(nc) as tc:
        with tc.tile_pool(name="const", bufs=1) as cpool, \
             tc.tile_pool(name="state", bufs=1) as spool, \
             tc.tile_pool(name="mm", bufs=4, space="PSUM") as mmp, \
             tc.tile_pool(name="tp", bufs=4, space="PSUM") as tpp:

            gts = cpool.tile([C2, NMM * 3 * C2], F32R, tag="gates")
            ident = cpool.tile([C2, C2], F32R, tag="ident")
            wread = cpool.tile([C2, 4], F32R, tag="wread")
            stA = spool.tile([C2, FTOT], F32R, tag="stA")
            stB = spool.tile([C2, FTOT], F32R, tag="stB")
            ptile = spool.tile([C2, PF], F32R, tag="probs")
            rtile = spool.tile([4, PF], F32, tag="rt")

            nc.sync.dma_start(out=stA[:, :], in_=d_state[:, :].bitcast(F32R))
            nc.sync.dma_start(out=ident[:, :], in_=d_ident[:, :].bitcast(F32R))
            nc.sync.dma_start(out=wread[:, :], in_=d_wread[:, :].bitcast(F32R))
            nc.sync.dma_start(out=gts[:, :], in_=d_gates[:, :].bitcast(F32R))

            drain_tgl = [0]

            def drain(dst_ap, src_ap):
                if drain_tgl[0] == 0:
                    nc.scalar.copy(out=dst_ap, in_=src_ap)
                else:
                    nc.vector.tensor_copy(dst_ap, src_ap)
                drain_tgl[0] ^= 1

            # swap flags per mm index (for drain AP chunking)
            mm_swap = []
            for l in range(LAYERS):
                mm_swap += [False, True, False, False, False, False,
                            False, True, False, False, False, False]
            mm_swap.append(False)

            cur, nxt = stA, stB
            mi = 0
            for op in FULL_OPS * repeats:
                if mi == NMM:
                    mi = 0  # timing-only extra passes reuse the gate stream
                if op == 'M':
                    swap = mm_swap[mi]
                    Ur = gts[:, mi * 300:mi * 300 + 100]
                    nUi = gts[:, mi * 300 + 100:mi * 300 + 200]
                    Ui = gts[:, mi * 300 + 200:mi * 300 + 300]
                    mi += 1
                    CH = 320          # aligned to the 160-wide shuffle slices
                    NCH = GF // CH
                    for g in range(NG):
                        for n in range(NCH):
                            fo = g * GF + n * CH
                            re_in = cur[:, fo:fo + CH]
                            im_in = cur[:, PF + fo:PF + fo + CH]
                            for plane in range(2):
                                ps = mmp.tile([C2, 400], F32, tag="mm")
                                w2m = nUi if plane == 0 else Ui
                                r1, r2 = (re_in, im_in) if plane == 0 else (im_in, re_in)
                                nc.tensor.matmul(ps[:, 0:CH], Ur, r1, start=True, stop=False)
                                nc.tensor.matmul(ps[:, 0:CH], w2m, r2, start=False, stop=True)
                                if not swap:
                                    dst = nxt[:, plane * PF + fo:plane * PF + fo + CH]
                                    src = ps[:, 0:CH].bitcast(F32R)
                                else:
                                    # psum chunk enumerates (fmaj:2, fmin:10, b:16);
                                    # swapped dst offset = fmin*160 + fmaj*16 + b
                                    base = plane * PF + g * GF
                                    dst = nxt[:, base:base + GF].rearrange(
                                        "p (fmin fmaj b) -> p fmaj fmin b",
                                        fmin=CUT, fmaj=CUT, b=BG)[:, 2 * n:2 * n + 2, :, :]
                                    src = ps[:, 0:CH].bitcast(F32R).rearrange(
                                        "p (fmaj fmin b) -> p fmaj fmin b",
                                        fmaj=2, fmin=CUT, b=BG)
                                drain(dst, src)
                elif op == 'S':
                    for g in range(NG):
                        for plane in range(2):
                            base = plane * PF + g * GF
                            for v in range(CUT):
                                s_ap = cur[v * CUT:(v + 1) * CUT, base:base + GF] \
                                    .rearrange("s (u r) -> s u r", u=CUT, r=CUT * BG)
                                d_ap = nxt[:, base + v * CUT * BG: base + (v + 1) * CUT * BG]
                                nc.sync.dma_start(out=d_ap, in_=s_ap)
                else:  # flip
                    for g in range(NG):
                        for plane in range(2):
                            base = plane * PF + g * GF
                            for bq in range(BG // 4):
                                pt = tpp.tile([C2, 400], F32R, tag="tp")
                                for q in range(4):
                                    bb = bq * 4 + q
                                    in_ap = cur[:, base:base + GF].rearrange(
                                        "p (f b) -> p f b", f=C2, b=BG)[:, :, bb]
                                    nc.tensor.transpose(pt[:, q * 100:(q + 1) * 100],
                                                        in_ap, ident[:, :])
                                # drain 4 transposes; dst free digits reversed:
                                # pt free = (b4, p1, p2); dst f = p2*160 + p1*16 + b
                                dstf = nxt[:, base:base + GF].rearrange(
                                    "p (p2 p1 b) -> p b p1 p2", p2=CUT, p1=CUT, b=BG)[:, bq * 4:bq * 4 + 4, :, :]
                                drain(dstf, pt[:, :].rearrange(
                                    "p (b p1 p2) -> p b p1 p2", b=4, p1=CUT, p2=CUT))
                cur, nxt = nxt, cur

            # readout: P = re^2 + im^2 (f32r so walrus rounds for the matmul)
            tmp = spool.tile([C2, PF], F32R, tag="probs2")
            nc.vector.tensor_mul(ptile[:, :], cur[:, 0:PF], cur[:, 0:PF])
            nc.vector.tensor_mul(tmp[:, :], cur[:, PF:FTOT], cur[:, PF:FTOT])
            nc.vector.tensor_add(ptile[:, :], ptile[:, :], tmp[:, :])
            for n in range(PF // 400):
                pr = mmp.tile([4, 400], F32, tag="mm")
                nc.tensor.matmul(pr[:, :], wread[:, :], ptile[:, n * 400:(n + 1) * 400],
                                 start=True, stop=True)
                drain(rtile[:, n * 400:(n + 1) * 400], pr[:, :])
            nc.sync.dma_start(out=d_rout[:, :], in_=rtile[:, :])

    nc.finalize()
    _legalize_waits(nc)
    _NC_CACHE[repeats] = nc
    return nc


def _legalize_waits(nc):
    """This walrus build encodes at most ONE sync wait per instruction.
    Split any instruction with N>1 waits into (N-1) preceding single-wait
    NoOps on the same engine (engines execute in order, so sequential
    waits are equivalent to simultaneous ones)."""
    import copy
    import concourse.mybir as mybir
    m = nc.m
    new_module = copy.replace(m, functions=[])
    nsplit = [0]
    for function in m.functions:
        new_function = copy.replace(function, blocks=[])
        new_function.set_allocations_from_list(function.allocations)
        for block in function.blocks:
            new_insts = []
            for inst in block.instructions:
                si = inst.sync_info
                if si is not None and si.on_wait and len(si.on_wait) > 1:
                    waits = list(si.on_wait)
                    for k, w in enumerate(waits[:-1]):
                        new_insts.append(mybir.InstNoOp(
                            name=f"{inst.name}-lw{k}",
                            engine=inst.engine,
                            sync_info=mybir.SyncInfo(on_wait=[w], on_update=[]),
                            bass_nofuse=True,
                        ))
                    inst.sync_info = mybir.SyncInfo(
                        on_wait=[waits[-1]], on_update=list(si.on_update))
                    nsplit[0] += 1
                new_insts.append(inst)
            new_function.blocks.append(copy.replace(block, instructions=new_insts))
        new_module.functions.append(new_function)
    nc.m = new_module
    return nsplit[0]


def kernel(inputs, cvqnn_weights, batch_size):
    inputs = np.asarray(inputs)
    assert inputs.shape[0] == int(batch_size) == B
    mms = build_gates(np.asarray(cvqnn_weights))
    st = initial_state_dev(inputs)
    gd = gates_dram(mms)
    ident = np.eye(C2, dtype=np.float32)
    wr = readout_weights()

    nc = build_bass()
    from concourse.bass_utils import run_bass_kernel_spmd
    in_maps = [{"state0": st[c], "gates": gd, "ident": ident, "wread": wr}
               for c in range(NCORES)]
    res = run_bass_kernel_spmd(nc, in_maps, core_ids=list(range(NCORES)))
    routs = [res.results[c]["rout"] for c in range(NCORES)]
    return assemble_output(routs)



# revision 12
# speedup vs baseline: 1.1708x; 1.1708x over previous
"""CVQNN batched policy forward on 8 Trainium2 NeuronCores (fp16 v2).

Math: B=256 4-mode Fock states (cutoff 10) through 4 CVQNN layers.
Device layout per core: state [100 partitions = mode pair (maj,min),
6400 free = fA*640 + plane*320 + fB*32 + b], fp16, complex as separate
re/im planes nested INSIDE each fA block so a pair-digit shuffle is a
contiguous 640-element move.

Every gate is a 100x100 complex matmul on the partition-side mode pair
(4 fp16 matmuls with PSUM accumulation, 640-wide chunks, 3 LDWEIGHTS
per gate via weight-reuse ordering). Diagonal/single-mode gates are
folded into pair gates on the host. Pair-layout changes: 'S' = 10
SBUF->SBUF DMAs (1280B descriptors), 'F' = PE transposes. Batch is
data-parallel over 8 cores.
"""
import numpy as np

B, D, CUT, LAYERS, NCORES = 256, 4, 10, 4, 8
K = D * (D - 1) // 2
C2 = CUT * CUT
BC = B // NCORES            # 32 batch per core
# free strides (elements): fA*640 + plane*320 + fB*32 + b
FTOT = 2 * C2 * BC          # 6400 free elems per state tile
PF = C2 * BC                # 3200 elems per plane (logical)
NMM = 12 * LAYERS + 1       # 49 pair-gate matmuls

# ---------------------------------------------------------------- host math

def _ops():
    a = np.diag(np.sqrt(np.arange(1, CUT)), k=1).astype(np.complex128)
    return a, a.conj().T.copy(), np.arange(CUT, dtype=np.float64)


def _expm_antiherm(G):
    w, V = np.linalg.eigh(-1j * G)
    return (V * np.exp(1j * w)) @ V.conj().T


def _bs(a, adag, t, p):
    Aab = np.kron(adag, a)
    return _expm_antiherm(t * (np.exp(1j * p) * Aab - np.exp(-1j * p) * Aab.conj().T))


def _orient(M, pair, in_order, out_order):
    """M acts canonically on index m_x*10+m_y for pair=(x,y).  Reindex for
    input digit order in_order and output digit order out_order."""
    M4 = M.reshape(CUT, CUT, CUT, CUT)  # [ox, oy, ix, iy]
    perm = [0, 1, 2, 3]
    if tuple(out_order) != tuple(pair):
        perm[0], perm[1] = perm[1], perm[0]
    if tuple(in_order) != tuple(pair):
        perm[2], perm[3] = perm[3], perm[2]
    return M4.transpose(perm).reshape(C2, C2)


def build_gates(cvqnn_weights):
    """Returns mm_list where entries are (matrix[100x100 complex],
    swap_free: bool). Layout walk is fixed."""
    a, adag, n = _ops()
    I10 = np.eye(CUT, dtype=np.complex128)
    w = np.asarray(cvqnn_weights, np.float64)
    PAIRS = [(0, 1), (0, 2), (0, 3), (1, 2), (1, 3), (2, 3)]

    mms = []
    fold01 = np.eye(C2, dtype=np.complex128)
    fold2 = I10.copy()
    fold3 = I10.copy()
    for l in range(LAYERS):
        o = 0
        th1 = w[l, o:o + K]; o += K
        ph1 = w[l, o:o + K]; o += K
        vp1 = w[l, o:o + D]; o += D
        rsq = w[l, o:o + D]; o += D
        th2 = w[l, o:o + K]; o += K
        ph2 = w[l, o:o + K]; o += K
        vp2 = w[l, o:o + D]; o += D
        rd = w[l, o:o + D]; o += D
        phd = w[l, o:o + D]; o += D
        kap = w[l, o:o + D]
        U = {PAIRS[q]: _bs(a, adag, th1[q], ph1[q]) for q in range(K)}
        V = {PAIRS[q]: _bs(a, adag, th2[q], ph2[q]) for q in range(K)}
        S = [_expm_antiherm(0.5 * rsq[m] * (a @ a - adag @ adag)) for m in range(D)]
        al = rd * np.exp(1j * phd)
        Dm = [_expm_antiherm(al[m] * adag - np.conj(al[m]) * a) for m in range(D)]
        P1 = [np.diag(np.exp(1j * vp1[m] * n)) for m in range(D)]
        P2 = [np.diag(np.exp(1j * vp2[m] * n)) for m in range(D)]
        Km = [np.diag(np.exp(1j * kap[m] * n * n)) for m in range(D)]
        SQ01 = np.kron(S[0] @ P1[0], S[1] @ P1[1])
        SQ23 = np.kron(S[2] @ P1[2], S[3] @ P1[3])
        DP01 = np.kron(Dm[0] @ P2[0], Dm[1] @ P2[1])
        DP23 = np.kron(Dm[2] @ P2[2], Dm[3] @ P2[3])

        mms.append((_orient(U[(0, 1)] @ fold01, (0, 1), (1, 0), (1, 0)), False))
        mms.append((_orient(U[(0, 2)] @ np.kron(I10, fold2), (0, 2), (0, 2), (2, 0)), True))
        mms.append((_orient(U[(0, 3)] @ np.kron(I10, fold3), (0, 3), (0, 3), (0, 3)), False))
        mms.append((_orient(U[(1, 2)], (1, 2), (2, 1), (2, 1)), False))
        mms.append((_orient(U[(1, 3)], (1, 3), (1, 3), (1, 3)), False))
        mms.append((_orient(SQ23 @ U[(2, 3)], (2, 3), (3, 2), (3, 2)), False))
        mms.append((_orient(V[(0, 1)] @ SQ01, (0, 1), (1, 0), (1, 0)), False))
        mms.append((_orient(V[(0, 2)], (0, 2), (0, 2), (2, 0)), True))
        mms.append((_orient(V[(0, 3)], (0, 3), (0, 3), (0, 3)), False))
        mms.append((_orient(V[(1, 2)], (1, 2), (2, 1), (2, 1)), False))
        mms.append((_orient(V[(1, 3)], (1, 3), (1, 3), (1, 3)), False))
        mms.append((_orient(DP23 @ V[(2, 3)], (2, 3), (3, 2), (3, 2)), False))
        if l < LAYERS - 1:
            fold01 = np.kron(Km[0], Km[1]) @ DP01
            fold2 = Km[2]
            fold3 = Km[3]
        else:
            mms.append((_orient(DP01, (0, 1), (1, 0), (1, 0)), False))
    assert len(mms) == NMM
    return mms


# op schedule per layer: 'M' gate, 'S' shuffle, 'F' flip
LAYER_OPS = ['M', 'S', 'M', 'S', 'M', 'F', 'M', 'S', 'M', 'S', 'M', 'F',
             'M', 'S', 'M', 'S', 'M', 'F', 'M', 'S', 'M', 'S', 'M', 'F']
FULL_OPS = LAYER_OPS * LAYERS + ['M']

MM_SWAP = []
for _l in range(LAYERS):
    MM_SWAP += [False, True, False, False, False, False,
                False, True, False, False, False, False]
MM_SWAP.append(False)


def initial_state_dev(inputs):
    """Per-core device state tiles [NCORES, 100, 6400] fp16 in layout
    [1,0 | 2,3]: p=m1*10+m0, f=fA(m2)*640 + plane*320 + fB(m3)*32 + b."""
    a, adag, n = _ops()
    z = 0.5j
    S0 = _expm_antiherm(0.5 * (np.conj(z) * (a @ a) - z * (adag @ adag)))
    psi0 = S0[:, 0]
    r = np.asarray(inputs, np.float64).reshape(-1)
    wv, Vx = np.linalg.eigh(-1j * (adag - a))
    w0 = Vx.conj().T @ psi0
    psi = (np.exp(1j * np.outer(r, wv)) * w0[None, :]) @ Vx.T
    psi = psi.reshape(B, D, CUT)
    st = np.einsum('bi,bj,bk,bl->bijkl', psi[:, 0], psi[:, 1], psi[:, 2], psi[:, 3])
    # [b, m0,m1,m2,m3] -> p=(m1,m0), fA=m2, fB=m3
    st = st.transpose(2, 1, 3, 4, 0).reshape(C2, CUT, CUT, B)  # [p, fA, fB, b]
    out = np.empty((NCORES, C2, CUT, 2, CUT, BC), np.float32)
    for c in range(NCORES):
        blk = st[:, :, :, c * BC:(c + 1) * BC]  # [100, 10, 10, 32]
        out[c, :, :, 0, :, :] = blk.real
        out[c, :, :, 1, :, :] = blk.imag
    return out.reshape(NCORES, C2, FTOT).astype(np.float16)


def readout_weights():
    """lhsT [100, 4] fp16 for the device readout matmul, layout [1,0|2,3]."""
    n = np.arange(CUT, dtype=np.float32)
    Wt = np.zeros((C2, 4), np.float32)
    for p in range(C2):
        Wt[p, 0] = n[p % 10]    # mode 0 (partition minor)
        Wt[p, 1] = n[p // 10]   # mode 1 (partition major)
        Wt[p, 2] = 1.0
        Wt[p, 3] = 1.0
    return Wt.astype(np.float16)


def assemble_output(routs):
    """routs: [NCORES, 4, 3200] f32 (free = fA*320 + fB*32 + b) -> [B, 4]."""
    n = np.arange(CUT, dtype=np.float64)
    out = np.zeros((B, D), np.float64)
    for c in range(NCORES):
        R = np.asarray(routs[c], np.float64).reshape(4, CUT, CUT, BC)
        sl = slice(c * BC, (c + 1) * BC)
        out[sl, 0] = R[0].sum(axis=(0, 1))
        out[sl, 1] = R[1].sum(axis=(0, 1))
        out[sl, 2] = (R[2] * n[:, None, None]).sum(axis=(0, 1))   # weight by fA=m2
        out[sl, 3] = (R[3] * n[None, :, None]).sum(axis=(0, 1))   # weight by fB=m3
    return out.astype(np.float32)


def gates_dram(mms):
    """[100, NMM*300] fp16: per gate UrT | (-Ui)T | UiT, packed columns."""
    g = np.empty((C2, NMM * 3 * C2), np.float32)
    for i, (M, _) in enumerate(mms):
        g[:, i * 300:i * 300 + C2] = M.real.T.astype(np.float32)
        g[:, i * 300 + C2:i * 300 + 2 * C2] = (-M.imag.T).astype(np.float32)
        g[:, i * 300 + 2 * C2:i * 300 + 3 * C2] = M.imag.T.astype(np.float32)
    return g.astype(np.float16)


def make_in_maps(st, gd):
    ident = np.eye(C2, dtype=np.float16)
    wr = readout_weights()
    return [{"state0": st[c], "gates": gd, "ident": ident, "wread": wr}
            for c in range(NCORES)]


# ------------------------------------------------------------ numpy dev-sim

def dev_sim(state_core, mms):
    """Numpy model of the device op stream for one core (fp16 rounding).
    state_core: [100, 6400] f16. Returns R [4, 3200] f32."""
    f64 = np.float64
    rnd = lambda x: x.astype(np.float16).astype(f64)
    X = state_core.astype(f64).reshape(C2, CUT, 2, CUT, BC)  # [p, fA, pl, fB, b]
    mi = 0
    for op in FULL_OPS:
        if op == 'M':
            M, swap = mms[mi]; mi += 1
            Mr = rnd(M.real); Mi = rnd(M.imag)
            re = X[:, :, 0]; im = X[:, :, 1]          # [p, fA, fB, b]
            re2 = np.tensordot(Mr, re, axes=(1, 0)) - np.tensordot(Mi, im, axes=(1, 0))
            im2 = np.tensordot(Mr, im, axes=(1, 0)) + np.tensordot(Mi, re, axes=(1, 0))
            re2 = rnd(re2); im2 = rnd(im2)
            if swap:
                re2 = re2.transpose(0, 2, 1, 3)
                im2 = im2.transpose(0, 2, 1, 3)
            X = np.stack([re2, im2], axis=2).copy()
        elif op == 'S':
            # [v,s | u, pl, w, b] -> [s,u | v, pl, w, b]
            X6 = X.reshape(CUT, CUT, CUT, 2, CUT, BC)  # [v, s, u, pl, w, b]
            X = X6.transpose(1, 2, 0, 3, 4, 5).reshape(C2, CUT, 2, CUT, BC)
        else:  # F
            # [p1,p2 | f1, pl, f2, b] -> [f1,f2 | p2, pl, p1, b]
            X6 = X.reshape(CUT, CUT, CUT, 2, CUT, BC)  # [p1, p2, f1, pl, f2, b]
            X = X6.transpose(2, 4, 1, 3, 0, 5).reshape(C2, CUT, 2, CUT, BC)
    P = X[:, :, 0] ** 2 + X[:, :, 1] ** 2              # [p, fA, fB, b]
    Wt = readout_weights().astype(f64)
    return np.tensordot(Wt.T, P.reshape(C2, PF), axes=(1, 0)).astype(np.float32)


# ------------------------------------------------------------- bass program

_NC_CACHE = {}


def build_bass():
    if 0 in _NC_CACHE:
        return _NC_CACHE[0]
    import concourse.bass as bass
    import concourse.mybir as mybir
    from concourse.tile import TileContext
    F32 = mybir.dt.float32
    F16 = mybir.dt.float16

    nc = bass.Bass()
    d_state = nc.dram_tensor("state0", [C2, FTOT], F16, kind="ExternalInput")
    d_gates = nc.dram_tensor("gates", [C2, NMM * 3 * C2], F16, kind="ExternalInput")
    d_ident = nc.dram_tensor("ident", [C2, C2], F16, kind="ExternalInput")
    d_wread = nc.dram_tensor("wread", [C2, 4], F16, kind="ExternalInput")
    d_rout = nc.dram_tensor("rout", [4, PF], F32, kind="ExternalOutput")

    with TileContext(nc) as tc:
        with tc.tile_pool(name="const", bufs=1) as cpool, \
             tc.tile_pool(name="state", bufs=1) as spool, \
             tc.tile_pool(name="mm", bufs=3, space="PSUM") as mmp, \
             tc.tile_pool(name="tp", bufs=2, space="PSUM") as tpp:

            gts = cpool.tile([C2, NMM * 3 * C2], F16, tag="gates")
            ident = cpool.tile([C2, C2], F16, tag="ident")
            wread = cpool.tile([C2, 4], F16, tag="wread")
            stA = spool.tile([C2, FTOT], F16, tag="stA")
            stB = spool.tile([C2, FTOT], F16, tag="stB")
            ptile = spool.tile([C2, PF], F16, tag="probs")
            tmp16 = spool.tile([C2, PF], F16, tag="probs2")
            rtile = spool.tile([4, PF], F32, tag="rt")

            nc.sync.dma_start(out=stA[:, :], in_=d_state[:, :])
            nc.sync.dma_start(out=ident[:, :], in_=d_ident[:, :])
            nc.sync.dma_start(out=wread[:, :], in_=d_wread[:, :])
            nc.sync.dma_start(out=gts[:, :], in_=d_gates[:, :])

            drain_tgl = [0]

            def drain(dst_ap, src_ap):
                if drain_tgl[0] == 0:
                    nc.scalar.copy(out=dst_ap, in_=src_ap)
                else:
                    nc.vector.tensor_copy(dst_ap, src_ap)
                drain_tgl[0] ^= 1

            cur, nxt = stA, stB
            mi = 0
            for oi, op in enumerate(FULL_OPS):
                # 5-d views [p, fA, pl, fB, b]
                cur5 = cur[:, :].rearrange("p (fA pl fB b) -> p fA pl fB b",
                                           fA=CUT, pl=2, fB=CUT, b=BC)
                nxt5 = nxt[:, :].rearrange("p (fA pl fB b) -> p fA pl fB b",
                                           fA=CUT, pl=2, fB=CUT, b=BC)
                # alternate layout (fA, fB, pl, b) used between a gate and its
                # following flip: (fA, fB) is a single stride-64 dim there, as
                # the PE transpose stationary operand requires
                curF = cur[:, :].rearrange("p (fA fB pl b) -> p fA fB pl b",
                                           fA=CUT, fB=CUT, pl=2, b=BC)
                nxtF = nxt[:, :].rearrange("p (fA fB pl b) -> p fA fB pl b",
                                           fA=CUT, fB=CUT, pl=2, b=BC)
                pre_flip = (op == 'M' and oi + 1 < len(FULL_OPS)
                            and FULL_OPS[oi + 1] == 'F')
                if op == 'M':
                    swap = MM_SWAP[mi]
                    Ur = gts[:, mi * 300:mi * 300 + 100]
                    nUi = gts[:, mi * 300 + 100:mi * 300 + 200]
                    Ui = gts[:, mi * 300 + 200:mi * 300 + 300]
                    mi += 1
                    for h in range(5):
                        # psum tiles span 2 banks: fA=2h at cols 0:320,
                        # fA=2h+1 at cols 512:832 (each within one bank)
                        psA = mmp.tile([C2, 1024], F32, tag="mm")
                        psB = mmp.tile([C2, 1024], F32, tag="mm")
                        rr = [cur5[:, 2 * h + k, 0, :, :] for k in range(2)]
                        ri = [cur5[:, 2 * h + k, 1, :, :] for k in range(2)]
                        # same-weight matmuls adjacent: 3 LDWEIGHTS per h
                        for k in range(2):
                            nc.tensor.matmul(psA[:, 512 * k:512 * k + 320], Ur,
                                             rr[k], start=True, stop=False)
                        for k in range(2):
                            nc.tensor.matmul(psB[:, 512 * k:512 * k + 320], Ur,
                                             ri[k], start=True, stop=False)
                        for k in range(2):
                            nc.tensor.matmul(psA[:, 512 * k:512 * k + 320], nUi,
                                             ri[k], start=False, stop=True)
                        for k in range(2):
                            nc.tensor.matmul(psB[:, 512 * k:512 * k + 320], Ui,
                                             rr[k], start=False, stop=True)
                        for pl, ps in ((0, psA), (1, psB)):
                            src = ps[:, :].rearrange("p (k r) -> p k r",
                                                     k=2, r=512)[:, :, 0:320] \
                                .rearrange("p k (j b) -> p k j b", j=CUT, b=BC)
                            if pre_flip:
                                dst = nxtF[:, 2 * h:2 * h + 2, :, pl, :]
                            elif not swap:
                                dst = nxt5[:, 2 * h:2 * h + 2, pl, :, :]
                            else:
                                # psum enum (k2, j10, b) -> dst fA=j, fB=2h+k
                                dst = nxt5[:, :, pl, 2 * h:2 * h + 2, :] \
                                    .rearrange("p j i b -> p i j b")
                            drain(dst, src)
                elif op == 'S':
                    for v in range(CUT):
                        eng = nc.sync if v < 6 else nc.gpsimd
                        eng.dma_start(
                            out=nxt[:, v * 640:(v + 1) * 640],
                            in_=cur[v * CUT:(v + 1) * CUT, :].rearrange(
                                "s (u r) -> s u r", u=CUT, r=640))
                else:  # F
                    for pl in range(2):
                        for bq in range(BC // 4):
                            pt = tpp.tile([C2, 400], F16, tag="tp")
                            for q in range(4):
                                bb = bq * 4 + q
                                in_ap = curF[:, :, :, pl, bb]   # [p, fA, fB]
                                nc.tensor.transpose(pt[:, q * 100:(q + 1) * 100],
                                                    in_ap, ident[:, :])
                            # psum enum (q, p1, p2) -> dst fA=p2, fB=p1, b=bq*4+q
                            dst = nxt5[:, :, pl, :, bq * 4:bq * 4 + 4] \
                                .rearrange("p p2 p1 q -> p q p1 p2")
                            src = pt[:, :].rearrange("p (q p1 p2) -> p q p1 p2",
                                                     q=4, p1=CUT, p2=CUT)
                            drain(dst, src)
                cur, nxt = nxt, cur

            # readout: P = re^2 + im^2
            cur5 = cur[:, :].rearrange("p (fA pl fB b) -> p fA pl fB b",
                                       fA=CUT, pl=2, fB=CUT, b=BC)
            re_ap = cur5[:, :, 0, :, :]
            im_ap = cur5[:, :, 1, :, :]
            pt3 = ptile[:, :].rearrange("p (fA fB b) -> p fA fB b",
                                        fA=CUT, fB=CUT, b=BC)
            tm3 = tmp16[:, :].rearrange("p (fA fB b) -> p fA fB b",
                                        fA=CUT, fB=CUT, b=BC)
            nc.vector.tensor_mul(pt3, re_ap, re_ap)
            nc.vector.tensor_mul(tm3, im_ap, im_ap)
            nc.vector.tensor_add(ptile[:, :], ptile[:, :], tmp16[:, :])
            for n in range(PF // 400):
                pr = tpp.tile([4, 400], F32, tag="tp")
                nc.tensor.matmul(pr[:, :], wread[:, :], ptile[:, n * 400:(n + 1) * 400],
                                 start=True, stop=True)
                drain(rtile[:, n * 400:(n + 1) * 400], pr[:, :])
            nc.sync.dma_start(out=d_rout[:, :], in_=rtile[:, :])

    nc.finalize()
    _legalize_waits(nc)
    _NC_CACHE[0] = nc
    return nc


def _legalize_waits(nc):
    """This walrus build encodes at most ONE sync wait per instruction.
    Split any instruction with N>1 waits into (N-1) preceding single-wait
    NoOps on the same engine (engines execute in order, so sequential
    waits are equivalent to simultaneous ones)."""
    import copy
    import concourse.mybir as mybir
    m = nc.m
    new_module = copy.replace(m, functions=[])
    nsplit = [0]
    for function in m.functions:
        new_function = copy.replace(function, blocks=[])
        new_function.set_allocations_from_list(function.allocations)
        for block in function.blocks:
            new_insts = []
            for inst in block.instructions:
                si = inst.sync_info
                if si is not None and si.on_wait and len(si.on_wait) > 1:
                    waits = list(si.on_wait)
                    for k, w in enumerate(waits[:-1]):
                        new_insts.append(mybir.InstNoOp(
                            name=f"{inst.name}-lw{k}",
                            engine=inst.engine,
                            sync_info=mybir.SyncInfo(on_wait=[w], on_update=[]),
                            bass_nofuse=True,
                        ))
                    inst.sync_info = mybir.SyncInfo(
                        on_wait=[waits[-1]], on_update=list(si.on_update))
                    nsplit[0] += 1
                new_insts.append(inst)
            new_function.blocks.append(copy.replace(block, instructions=new_insts))
        new_module.functions.append(new_function)
    nc.m = new_module
    return nsplit[0]


def kernel(inputs, cvqnn_weights, batch_size):
    inputs = np.asarray(inputs)
    assert inputs.shape[0] == int(batch_size) == B
    mms = build_gates(np.asarray(cvqnn_weights))
    st = initial_state_dev(inputs)
    gd = gates_dram(mms)

    nc = build_bass()
    from concourse.bass_utils import run_bass_kernel_spmd
    in_maps = make_in_maps(st, gd)
    res = run_bass_kernel_spmd(nc, in_maps, core_ids=list(range(NCORES)))
    routs = [res.results[c]["rout"] for c in range(NCORES)]
    return assemble_output(routs)


# revision 20
# speedup vs baseline: 1.4637x; 1.2501x over previous
"""CVQNN batched policy forward on 8 Trainium2 NeuronCores (fp16 v2).

Math: B=256 4-mode Fock states (cutoff 10) through 4 CVQNN layers.
Device layout per core: state [100 partitions = mode pair (maj,min),
6400 free = fA*640 + plane*320 + fB*32 + b], fp16, complex as separate
re/im planes nested INSIDE each fA block so a pair-digit shuffle is a
contiguous 640-element move.

Every gate is a 100x100 complex matmul on the partition-side mode pair
(4 fp16 matmuls with PSUM accumulation, 640-wide chunks, 3 LDWEIGHTS
per gate via weight-reuse ordering). Diagonal/single-mode gates are
folded into pair gates on the host. Pair-layout changes: 'S' = 10
SBUF->SBUF DMAs (1280B descriptors), 'F' = PE transposes. Batch is
data-parallel over 8 cores.
"""
import numpy as np

B, D, CUT, LAYERS, NCORES = 256, 4, 10, 4, 8
K = D * (D - 1) // 2
C2 = CUT * CUT
BC = B // NCORES            # 32 batch per core
# free strides (elements): fA*640 + plane*320 + fB*32 + b
FTOT = 2 * C2 * BC          # 6400 free elems per state tile
PF = C2 * BC                # 3200 elems per plane (logical)
NMM = 12 * LAYERS + 1       # 49 pair-gate matmuls

# ---------------------------------------------------------------- host math

def _ops():
    a = np.diag(np.sqrt(np.arange(1, CUT)), k=1).astype(np.complex128)
    return a, a.conj().T.copy(), np.arange(CUT, dtype=np.float64)


def _expm_antiherm(G):
    w, V = np.linalg.eigh(-1j * G)
    return (V * np.exp(1j * w)) @ V.conj().T


def _bs(a, adag, t, p):
    Aab = np.kron(adag, a)
    return _expm_antiherm(t * (np.exp(1j * p) * Aab - np.exp(-1j * p) * Aab.conj().T))


def _orient(M, pair, in_order, out_order):
    """M acts canonically on index m_x*10+m_y for pair=(x,y).  Reindex for
    input digit order in_order and output digit order out_order."""
    M4 = M.reshape(CUT, CUT, CUT, CUT)  # [ox, oy, ix, iy]
    perm = [0, 1, 2, 3]
    if tuple(out_order) != tuple(pair):
        perm[0], perm[1] = perm[1], perm[0]
    if tuple(in_order) != tuple(pair):
        perm[2], perm[3] = perm[3], perm[2]
    return M4.transpose(perm).reshape(C2, C2)


def build_gates(cvqnn_weights):
    """Returns mm_list where entries are (matrix[100x100 complex],
    swap_free: bool). Layout walk is fixed."""
    a, adag, n = _ops()
    I10 = np.eye(CUT, dtype=np.complex128)
    w = np.asarray(cvqnn_weights, np.float64)
    PAIRS = [(0, 1), (0, 2), (0, 3), (1, 2), (1, 3), (2, 3)]

    mms = []
    fold01 = np.eye(C2, dtype=np.complex128)
    fold2 = I10.copy()
    fold3 = I10.copy()
    for l in range(LAYERS):
        o = 0
        th1 = w[l, o:o + K]; o += K
        ph1 = w[l, o:o + K]; o += K
        vp1 = w[l, o:o + D]; o += D
        rsq = w[l, o:o + D]; o += D
        th2 = w[l, o:o + K]; o += K
        ph2 = w[l, o:o + K]; o += K
        vp2 = w[l, o:o + D]; o += D
        rd = w[l, o:o + D]; o += D
        phd = w[l, o:o + D]; o += D
        kap = w[l, o:o + D]
        U = {PAIRS[q]: _bs(a, adag, th1[q], ph1[q]) for q in range(K)}
        V = {PAIRS[q]: _bs(a, adag, th2[q], ph2[q]) for q in range(K)}
        S = [_expm_antiherm(0.5 * rsq[m] * (a @ a - adag @ adag)) for m in range(D)]
        al = rd * np.exp(1j * phd)
        Dm = [_expm_antiherm(al[m] * adag - np.conj(al[m]) * a) for m in range(D)]
        P1 = [np.diag(np.exp(1j * vp1[m] * n)) for m in range(D)]
        P2 = [np.diag(np.exp(1j * vp2[m] * n)) for m in range(D)]
        Km = [np.diag(np.exp(1j * kap[m] * n * n)) for m in range(D)]
        SQ01 = np.kron(S[0] @ P1[0], S[1] @ P1[1])
        SQ23 = np.kron(S[2] @ P1[2], S[3] @ P1[3])
        DP01 = np.kron(Dm[0] @ P2[0], Dm[1] @ P2[1])
        DP23 = np.kron(Dm[2] @ P2[2], Dm[3] @ P2[3])

        mms.append((_orient(U[(0, 1)] @ fold01, (0, 1), (1, 0), (1, 0)), False))
        mms.append((_orient(U[(0, 2)] @ np.kron(I10, fold2), (0, 2), (0, 2), (2, 0)), True))
        mms.append((_orient(U[(0, 3)] @ np.kron(I10, fold3), (0, 3), (0, 3), (0, 3)), False))
        mms.append((_orient(U[(1, 2)], (1, 2), (2, 1), (2, 1)), False))
        mms.append((_orient(U[(1, 3)], (1, 3), (1, 3), (1, 3)), False))
        mms.append((_orient(SQ23 @ U[(2, 3)], (2, 3), (3, 2), (3, 2)), False))
        mms.append((_orient(V[(0, 1)] @ SQ01, (0, 1), (1, 0), (1, 0)), False))
        mms.append((_orient(V[(0, 2)], (0, 2), (0, 2), (2, 0)), True))
        mms.append((_orient(V[(0, 3)], (0, 3), (0, 3), (0, 3)), False))
        mms.append((_orient(V[(1, 2)], (1, 2), (2, 1), (2, 1)), False))
        mms.append((_orient(V[(1, 3)], (1, 3), (1, 3), (1, 3)), False))
        mms.append((_orient(DP23 @ V[(2, 3)], (2, 3), (3, 2), (3, 2)), False))
        if l < LAYERS - 1:
            fold01 = np.kron(Km[0], Km[1]) @ DP01
            fold2 = Km[2]
            fold3 = Km[3]
        else:
            mms.append((_orient(DP01, (0, 1), (1, 0), (1, 0)), False))
    assert len(mms) == NMM
    return mms


# op schedule per layer: 'M' gate, 'S' shuffle, 'F' flip
LAYER_OPS = ['M', 'S', 'M', 'S', 'M', 'F', 'M', 'S', 'M', 'S', 'M', 'F',
             'M', 'S', 'M', 'S', 'M', 'F', 'M', 'S', 'M', 'S', 'M', 'F']
FULL_OPS = LAYER_OPS * LAYERS + ['M']

MM_SWAP = []
for _l in range(LAYERS):
    MM_SWAP += [False, True, False, False, False, False,
                False, True, False, False, False, False]
MM_SWAP.append(False)


def initial_state_dev(inputs):
    """Per-core device state tiles [NCORES, 100, 6400] fp16 in layout
    [1,0 | 2,3]: p=m1*10+m0, f=fA(m2)*640 + plane*320 + fB(m3)*32 + b."""
    a, adag, n = _ops()
    z = 0.5j
    S0 = _expm_antiherm(0.5 * (np.conj(z) * (a @ a) - z * (adag @ adag)))
    psi0 = S0[:, 0]
    r = np.asarray(inputs, np.float64).reshape(-1)
    wv, Vx = np.linalg.eigh(-1j * (adag - a))
    w0 = Vx.conj().T @ psi0
    psi = (np.exp(1j * np.outer(r, wv)) * w0[None, :]) @ Vx.T
    psi = psi.reshape(B, D, CUT)
    st = np.einsum('bi,bj,bk,bl->bijkl', psi[:, 0], psi[:, 1], psi[:, 2], psi[:, 3])
    # [b, m0,m1,m2,m3] -> p=(m1,m0), fA=m2, fB=m3
    st = st.transpose(2, 1, 3, 4, 0).reshape(C2, CUT, CUT, B)  # [p, fA, fB, b]
    out = np.empty((NCORES, C2, CUT, 2, CUT, BC), np.float32)
    for c in range(NCORES):
        blk = st[:, :, :, c * BC:(c + 1) * BC]  # [100, 10, 10, 32]
        out[c, :, :, 0, :, :] = blk.real
        out[c, :, :, 1, :, :] = blk.imag
    return out.reshape(NCORES, C2, FTOT).astype(np.float16)


def readout_weights():
    """lhsT [100, 4] fp16 for the device readout matmul, layout [1,0|2,3]."""
    n = np.arange(CUT, dtype=np.float32)
    Wt = np.zeros((C2, 4), np.float32)
    for p in range(C2):
        Wt[p, 0] = n[p % 10]    # mode 0 (partition minor)
        Wt[p, 1] = n[p // 10]   # mode 1 (partition major)
        Wt[p, 2] = 1.0
        Wt[p, 3] = 1.0
    return Wt.astype(np.float16)


def assemble_output(routs):
    """routs: [NCORES, 4, 3200] f32 (free = fA*320 + fB*32 + b) -> [B, 4]."""
    n = np.arange(CUT, dtype=np.float64)
    out = np.zeros((B, D), np.float64)
    for c in range(NCORES):
        R = np.asarray(routs[c], np.float64).reshape(4, CUT, CUT, BC)
        sl = slice(c * BC, (c + 1) * BC)
        out[sl, 0] = R[0].sum(axis=(0, 1))
        out[sl, 1] = R[1].sum(axis=(0, 1))
        out[sl, 2] = (R[2] * n[:, None, None]).sum(axis=(0, 1))   # weight by fA=m2
        out[sl, 3] = (R[3] * n[None, :, None]).sum(axis=(0, 1))   # weight by fB=m3
    return out.astype(np.float32)


GW = 128   # gate stationary padded to 128 cols (fast weight load)


def gates_dram(mms):
    """[100, NMM*3*GW] fp16: per gate UrT | (-Ui)T | UiT, each zero-padded
    to 128 columns so LDWEIGHTS takes the fast path."""
    g = np.zeros((C2, NMM * 3 * GW), np.float32)
    for i, (M, _) in enumerate(mms):
        g[:, i * 3 * GW:i * 3 * GW + C2] = M.real.T.astype(np.float32)
        g[:, i * 3 * GW + GW:i * 3 * GW + GW + C2] = (-M.imag.T).astype(np.float32)
        g[:, i * 3 * GW + 2 * GW:i * 3 * GW + 2 * GW + C2] = M.imag.T.astype(np.float32)
    return g.astype(np.float16)


def make_in_maps(st, gd):
    ident = np.eye(C2, dtype=np.float16)
    wr = readout_weights()
    return [{"state0": st[c], "gates": gd, "ident": ident, "wread": wr}
            for c in range(NCORES)]


# ------------------------------------------------------------ numpy dev-sim

def dev_sim(state_core, mms):
    """Numpy model of the device op stream for one core (fp16 rounding).
    state_core: [100, 6400] f16. Returns R [4, 3200] f32."""
    f64 = np.float64
    rnd = lambda x: x.astype(np.float16).astype(f64)
    X = state_core.astype(f64).reshape(C2, CUT, 2, CUT, BC)  # [p, fA, pl, fB, b]
    mi = 0
    for op in FULL_OPS:
        if op == 'M':
            M, swap = mms[mi]; mi += 1
            Mr = rnd(M.real); Mi = rnd(M.imag)
            re = X[:, :, 0]; im = X[:, :, 1]          # [p, fA, fB, b]
            re2 = np.tensordot(Mr, re, axes=(1, 0)) - np.tensordot(Mi, im, axes=(1, 0))
            im2 = np.tensordot(Mr, im, axes=(1, 0)) + np.tensordot(Mi, re, axes=(1, 0))
            re2 = rnd(re2); im2 = rnd(im2)
            if swap:
                re2 = re2.transpose(0, 2, 1, 3)
                im2 = im2.transpose(0, 2, 1, 3)
            X = np.stack([re2, im2], axis=2).copy()
        elif op == 'S':
            # [v,s | u, pl, w, b] -> [s,u | v, pl, w, b]
            X6 = X.reshape(CUT, CUT, CUT, 2, CUT, BC)  # [v, s, u, pl, w, b]
            X = X6.transpose(1, 2, 0, 3, 4, 5).reshape(C2, CUT, 2, CUT, BC)
        else:  # F
            # [p1,p2 | f1, pl, f2, b] -> [f1,f2 | p2, pl, p1, b]
            X6 = X.reshape(CUT, CUT, CUT, 2, CUT, BC)  # [p1, p2, f1, pl, f2, b]
            X = X6.transpose(2, 4, 1, 3, 0, 5).reshape(C2, CUT, 2, CUT, BC)
    P = X[:, :, 0] ** 2 + X[:, :, 1] ** 2              # [p, fA, fB, b]
    Wt = readout_weights().astype(f64)
    return np.tensordot(Wt.T, P.reshape(C2, PF), axes=(1, 0)).astype(np.float32)


# ------------------------------------------------------------- bass program

_NC_CACHE = {}


def build_bass():
    if 0 in _NC_CACHE:
        return _NC_CACHE[0]
    import concourse.bass as bass
    import concourse.mybir as mybir
    from concourse.tile import TileContext
    F32 = mybir.dt.float32
    F16 = mybir.dt.float16

    nc = bass.Bass()
    d_state = nc.dram_tensor("state0", [C2, FTOT], F16, kind="ExternalInput")
    d_gates = nc.dram_tensor("gates", [C2, NMM * 3 * GW], F16, kind="ExternalInput")
    d_ident = nc.dram_tensor("ident", [C2, C2], F16, kind="ExternalInput")
    d_wread = nc.dram_tensor("wread", [C2, 4], F16, kind="ExternalInput")
    d_rout = nc.dram_tensor("rout", [4, PF], F32, kind="ExternalOutput")

    with TileContext(nc) as tc:
        with tc.tile_pool(name="const", bufs=1) as cpool, \
             tc.tile_pool(name="state", bufs=1) as spool, \
             tc.tile_pool(name="mm", bufs=3, space="PSUM") as mmp, \
             tc.tile_pool(name="tp", bufs=2, space="PSUM") as tpp:

            gts = cpool.tile([C2, NMM * 3 * GW], F16, tag="gates")
            ident = cpool.tile([C2, C2], F16, tag="ident")
            wread = cpool.tile([C2, 4], F16, tag="wread")
            # +128 tail: flip stationary reads [off:off+128] and may spill
            # past the live 6400 elems (garbage rows >=100 are never drained)
            stA_t = spool.tile([C2, FTOT + 128], F16, tag="stA")
            stB_t = spool.tile([C2, FTOT + 128], F16, tag="stB")
            stA = stA_t[:, 0:FTOT]
            stB = stB_t[:, 0:FTOT]
            ptile = spool.tile([C2, PF], F16, tag="probs")
            tmp16 = spool.tile([C2, PF], F16, tag="probs2")
            rtile = spool.tile([4, PF], F32, tag="rt")

            nc.sync.dma_start(out=stA[:, :], in_=d_state[:, :])
            nc.sync.dma_start(out=ident[:, :], in_=d_ident[:, :])
            nc.sync.dma_start(out=wread[:, :], in_=d_wread[:, :])
            nc.sync.dma_start(out=gts[:, :], in_=d_gates[:, :])

            drain_tgl = [0]

            def drain(dst_ap, src_ap):
                if drain_tgl[0] == 0:
                    nc.scalar.copy(out=dst_ap, in_=src_ap)
                else:
                    nc.vector.tensor_copy(dst_ap, src_ap)
                drain_tgl[0] ^= 1

            cur_t, nxt_t = stA_t, stB_t
            mi = 0
            for oi, op in enumerate(FULL_OPS):
                cur = cur_t[:, 0:FTOT]
                nxt = nxt_t[:, 0:FTOT]
                # 5-d views [p, fA, pl, fB, b]
                cur5 = cur.rearrange("p (fA pl fB b) -> p fA pl fB b",
                                     fA=CUT, pl=2, fB=CUT, b=BC)
                nxt5 = nxt.rearrange("p (fA pl fB b) -> p fA pl fB b",
                                     fA=CUT, pl=2, fB=CUT, b=BC)
                # alternate layout (pl, b, fA, fB) used between a gate and its
                # following flip: the (fA, fB) block is 100 contiguous elems,
                # so the flip matmul's stationary operand is a plain slice
                nxtF = nxt.rearrange("p (pl b fA fB) -> p pl b fA fB",
                                     pl=2, b=BC, fA=CUT, fB=CUT)
                pre_flip = (op == 'M' and oi + 1 < len(FULL_OPS)
                            and FULL_OPS[oi + 1] == 'F')
                if op == 'M':
                    swap = MM_SWAP[mi]
                    Ur = gts[:, mi * 3 * GW:mi * 3 * GW + GW]
                    nUi = gts[:, mi * 3 * GW + GW:mi * 3 * GW + 2 * GW]
                    Ui = gts[:, mi * 3 * GW + 2 * GW:mi * 3 * GW + 3 * GW]
                    mi += 1
                    for h in range(5):
                        # psum tiles span 2 banks: fA=2h at cols 0:320,
                        # fA=2h+1 at cols 512:832 (each within one bank)
                        psA = mmp.tile([128, 1024], F32, tag="mm")
                        psB = mmp.tile([128, 1024], F32, tag="mm")
                        rr = [cur5[:, 2 * h + k, 0, :, :] for k in range(2)]
                        ri = [cur5[:, 2 * h + k, 1, :, :] for k in range(2)]
                        # same-weight matmuls adjacent: 3 LDWEIGHTS per h
                        for k in range(2):
                            nc.tensor.matmul(psA[:, 512 * k:512 * k + 320], Ur,
                                             rr[k], start=True, stop=False)
                        for k in range(2):
                            nc.tensor.matmul(psB[:, 512 * k:512 * k + 320], Ur,
                                             ri[k], start=True, stop=False)
                        for k in range(2):
                            nc.tensor.matmul(psA[:, 512 * k:512 * k + 320], nUi,
                                             ri[k], start=False, stop=True)
                        for k in range(2):
                            nc.tensor.matmul(psB[:, 512 * k:512 * k + 320], Ui,
                                             rr[k], start=False, stop=True)
                        for pl, ps in ((0, psA), (1, psB)):
                            src = ps[0:C2, :].rearrange("p (k r) -> p k r",
                                                        k=2, r=512)[:, :, 0:320] \
                                .rearrange("p k (j b) -> p k j b", j=CUT, b=BC)
                            if pre_flip:
                                # L*2 layout: dst iter (k, b, j)
                                src = ps[0:C2, :].rearrange("p (k r) -> p k r",
                                                            k=2, r=512)[:, :, 0:320] \
                                    .rearrange("p k (j b) -> p k b j", j=CUT, b=BC)
                                dst = nxtF[:, pl, :, 2 * h:2 * h + 2, :] \
                                    .rearrange("p b k j -> p k b j")
                            elif not swap:
                                dst = nxt5[:, 2 * h:2 * h + 2, pl, :, :]
                            else:
                                # psum enum (k2, j10, b) -> dst fA=j, fB=2h+k
                                dst = nxt5[:, :, pl, 2 * h:2 * h + 2, :] \
                                    .rearrange("p j i b -> p i j b")
                            drain(dst, src)
                elif op == 'S':
                    for v in range(CUT):
                        eng = nc.sync if v % 2 == 0 else nc.gpsimd
                        eng.dma_start(
                            out=nxt[:, v * 640:(v + 1) * 640],
                            in_=cur[v * CUT:(v + 1) * CUT, :].rearrange(
                                "s (u r) -> s u r", u=CUT, r=640))
                else:  # F
                    # flip via REGULAR matmul: stationary = contiguous 128-col
                    # slice of the L*2 state (100 live cols = (fA,fB) block),
                    # moving = fp16 identity.  out = slice.T @ I, partitions
                    # 100..127 are spill garbage and are never drained.
                    for pl in range(2):
                        for bq in range(BC // 4):
                            pt = tpp.tile([128, 400], F32, tag="tp")
                            for q in range(4):
                                bb = bq * 4 + q
                                off = pl * 3200 + bb * 100
                                lhsT = cur_t[:, off:off + 128]
                                nc.tensor.matmul(pt[:, q * 100:(q + 1) * 100],
                                                 lhsT, ident[:, :],
                                                 start=True, stop=True)
                            # psum enum (q, p1, p2) -> dst fA=p2, fB=p1,
                            # b=bq*4+q; iterate (p2, p1, q) so dst inner is
                            # the contiguous b-quad
                            dst = nxt5[:, :, pl, :, bq * 4:bq * 4 + 4]
                            src = pt[0:C2, :].rearrange("p (q p1 p2) -> p p2 p1 q",
                                                        q=4, p1=CUT, p2=CUT)
                            drain(dst, src)
                cur_t, nxt_t = nxt_t, cur_t

            # readout: P = re^2 + im^2
            cur = cur_t[:, 0:FTOT]
            cur5 = cur.rearrange("p (fA pl fB b) -> p fA pl fB b",
                                 fA=CUT, pl=2, fB=CUT, b=BC)
            re_ap = cur5[:, :, 0, :, :]
            im_ap = cur5[:, :, 1, :, :]
            pt3 = ptile[:, :].rearrange("p (fA fB b) -> p fA fB b",
                                        fA=CUT, fB=CUT, b=BC)
            tm3 = tmp16[:, :].rearrange("p (fA fB b) -> p fA fB b",
                                        fA=CUT, fB=CUT, b=BC)
            nc.vector.tensor_mul(pt3, re_ap, re_ap)
            nc.vector.tensor_mul(tm3, im_ap, im_ap)
            nc.vector.tensor_add(ptile[:, :], ptile[:, :], tmp16[:, :])
            for n in range(PF // 400):
                pr = tpp.tile([4, 400], F32, tag="tp")
                nc.tensor.matmul(pr[:, :], wread[:, :], ptile[:, n * 400:(n + 1) * 400],
                                 start=True, stop=True)
                drain(rtile[:, n * 400:(n + 1) * 400], pr[:, :])
            nc.sync.dma_start(out=d_rout[:, :], in_=rtile[:, :])

    nc.finalize()
    _legalize_waits(nc)
    _NC_CACHE[0] = nc
    return nc


def _legalize_waits(nc):
    """This walrus build encodes at most ONE sync wait per instruction.
    Split any instruction with N>1 waits into (N-1) preceding single-wait
    NoOps on the same engine (engines execute in order, so sequential
    waits are equivalent to simultaneous ones)."""
    import copy
    import concourse.mybir as mybir
    m = nc.m
    new_module = copy.replace(m, functions=[])
    nsplit = [0]
    for function in m.functions:
        new_function = copy.replace(function, blocks=[])
        new_function.set_allocations_from_list(function.allocations)
        for block in function.blocks:
            new_insts = []
            for inst in block.instructions:
                si = inst.sync_info
                if si is not None and si.on_wait and len(si.on_wait) > 1:
                    waits = list(si.on_wait)
                    for k, w in enumerate(waits[:-1]):
                        new_insts.append(mybir.InstNoOp(
                            name=f"{inst.name}-lw{k}",
                            engine=inst.engine,
                            sync_info=mybir.SyncInfo(on_wait=[w], on_update=[]),
                            bass_nofuse=True,
                        ))
                    inst.sync_info = mybir.SyncInfo(
                        on_wait=[waits[-1]], on_update=list(si.on_update))
                    nsplit[0] += 1
                new_insts.append(inst)
            new_function.blocks.append(copy.replace(block, instructions=new_insts))
        new_module.functions.append(new_function)
    nc.m = new_module
    return nsplit[0]


def kernel(inputs, cvqnn_weights, batch_size):
    inputs = np.asarray(inputs)
    assert inputs.shape[0] == int(batch_size) == B
    mms = build_gates(np.asarray(cvqnn_weights))
    st = initial_state_dev(inputs)
    gd = gates_dram(mms)

    nc = build_bass()
    from concourse.bass_utils import run_bass_kernel_spmd
    in_maps = make_in_maps(st, gd)
    res = run_bass_kernel_spmd(nc, in_maps, core_ids=list(range(NCORES)))
    routs = [res.results[c]["rout"] for c in range(NCORES)]
    return assemble_output(routs)
